# revision 2
# baseline (speedup 1.0000x reference)
"""Multi-head attention block (B=16, N=1024, D=768, H=12) on 8 TRN2 NeuronCores.

Strategy: pure data parallelism — 2 batch items per core, no collectives.
Host pre-transposes x to x^T and casts operands to bf16, so the device
needs no transposes at all:
  - QKV for q,k computed in transposed orientation (qkT [1536, rows]):
    lhsT = W_qkv tile, rhs = x^T tile.
  - v computed in natural orientation [rows, 768] (PV lhsT wants it):
    lhsT = x^T tile, rhs = W_qkv v-columns; a ones column is appended
    per head so the PV matmul also produces the softmax denominators.
  - scores computed transposed [keys, rows] (softmax'd probabilities are
    then directly the PV moving operand). K=64 per head; heads are
    processed in pairs at base partitions 0/64 so the two matmuls pack
    into distinct PE row groups and run concurrently.
  - exp on ScalarE with the 1/sqrt(hd) scale fused; no max subtraction
    (scores are ~N(0,1) by construction, exp cannot overflow).
  - attention output accumulates as attn_out^T [768, rows], which is
    exactly the lhsT layout the output projection needs.
  - softmax normalization: reciprocal of the ones-row sums, gpsimd
    partition-broadcast, fused into the PSUM->SBUF copyback multiply.
"""

import sys
import types
import numpy as np
import ml_dtypes
from contextlib import ExitStack

# --- shim: provide antenv.axon_hooks so trace=True works under axon ---
if "antenv.axon_hooks" not in sys.modules:
    try:
        from trn_agent_boot.trn_boot import _ntff_profile_via_ctypes

        _hooks_mod = types.ModuleType("antenv.axon_hooks")
        _ntff_hook = _ntff_profile_via_ctypes("/opt/axon/libaxon_pjrt.so")
        _hooks_mod.get_axon_ntff_profile_hook = lambda: _ntff_hook
        _hooks_mod.set_axon_ntff_profile_hook = lambda h: None
        sys.modules["antenv.axon_hooks"] = _hooks_mod
    except Exception:
        pass

import concourse.bass as bass
import concourse.tile as tile
from concourse import bacc, mybir
import concourse.bass_utils as bass_utils
from concourse.bass_utils import run_bass_kernel_spmd

bass_utils.upload_artifacts = lambda tmpdir: tmpdir  # no S3 in sandbox

F32 = mybir.dt.float32
BF16 = mybir.dt.bfloat16
EXP = mybir.ActivationFunctionType.Exp

NCORES = 8
B, N, D = 16, 1024, 768
H, HD = 12, 64
BPC = B // NCORES        # batch items per core
ROWS = BPC * N           # 2048
P = 128
KT = D // P              # 6 contraction tiles
SCALE = HD ** -0.5


def build_kernel():
    nc = bacc.Bacc("TRN2", target_bir_lowering=False, debug=False, num_devices=NCORES)
    xT = nc.dram_tensor("xT", [D, ROWS], BF16, kind="ExternalInput").ap()
    wqkv = nc.dram_tensor("wqkv", [D, 3 * D], BF16, kind="ExternalInput").ap()
    wproj = nc.dram_tensor("wproj", [D, D], BF16, kind="ExternalInput").ap()
    bias = nc.dram_tensor("bias", [P, D], F32, kind="ExternalInput").ap()
    out = nc.dram_tensor("out", [ROWS, D], F32, kind="ExternalOutput").ap()

    with tile.TileContext(nc) as tc, ExitStack() as ctx:
        const = ctx.enter_context(tc.tile_pool(name="const", bufs=1))
        xp = ctx.enter_context(tc.tile_pool(name="xT", bufs=2))
        qkp = ctx.enter_context(tc.tile_pool(name="qkT", bufs=2))
        vp = ctx.enter_context(tc.tile_pool(name="v", bufs=2))
        aop = ctx.enter_context(tc.tile_pool(name="ao", bufs=2))
        exp_p = ctx.enter_context(tc.tile_pool(name="exp", bufs=3))
        yp = ctx.enter_context(tc.tile_pool(name="y", bufs=3))
        smallp = ctx.enter_context(tc.tile_pool(name="small", bufs=3))
        ps_mm = ctx.enter_context(tc.tile_pool(name="ps_mm", bufs=2, space="PSUM"))
        ps_sc = ctx.enter_context(tc.tile_pool(name="ps_sc", bufs=2, space="PSUM"))
        ps_out = ctx.enter_context(tc.tile_pool(name="ps_out", bufs=2, space="PSUM"))

        # resident weights
        wqkv_sb = const.tile([P, KT, 3 * D], BF16)
        nc.sync.dma_start(wqkv_sb[:], wqkv.rearrange("(a p) n -> p a n", p=P))
        wproj_sb = const.tile([P, KT, D], BF16)
        nc.sync.dma_start(wproj_sb[:], wproj.rearrange("(a p) n -> p a n", p=P))
        bias_sb = const.tile([P, D], F32)
        nc.sync.dma_start(bias_sb[:], bias)

        for b in range(BPC):
            rows0 = b * N
            xT_t = xp.tile([P, KT, N], BF16)
            nc.sync.dma_start(
                xT_t[:], xT[:, rows0:rows0 + N].rearrange("(a p) n -> p a n", p=P)
            )

            # ---- QKV: q,k transposed [1536, N] ----
            qkT_t = qkp.tile([P, 2 * KT, N], BF16)
            for nt in range(2 * KT):
                for hf in range(2):
                    pm = ps_mm.tile([P, 512], F32)
                    for kt in range(KT):
                        nc.tensor.matmul(
                            pm[:],
                            wqkv_sb[:, kt, nt * P:(nt + 1) * P],
                            xT_t[:, kt, hf * 512:(hf + 1) * 512],
                            start=(kt == 0), stop=(kt == KT - 1),
                        )
                    nc.vector.tensor_copy(qkT_t[:, nt, hf * 512:(hf + 1) * 512], pm[:])

            # ---- V natural [N, 768] with appended ones column per head ----
            v_t = vp.tile([P, N // P, H, HD + 1], BF16)
            nc.vector.memset(v_t[:, :, :, HD:HD + 1], 1.0)
            for rt in range(N // P):
                for j in range(2):
                    pm = ps_mm.tile([P, 512], F32)
                    for kt in range(KT):
                        nc.tensor.matmul(
                            pm[:, :384],
                            xT_t[:, kt, rt * P:(rt + 1) * P],
                            wqkv_sb[:, kt, 2 * D + j * 384:2 * D + (j + 1) * 384],
                            start=(kt == 0), stop=(kt == KT - 1),
                        )
                    nc.vector.tensor_copy(v_t[:, rt, j * 6:(j + 1) * 6, 0:HD], pm[:, :384])

            # ---- attention, head pairs, transposed scores ----
            ao_t = aop.tile([P, KT, N], BF16)
            for p in range(H // 2):
                for hf in range(2):
                    po = [
                        ps_out.tile([HD + 1, 512], F32, tag="po", name=f"po_{b}_{p}_{hf}_{hs}")
                        for hs in range(2)
                    ]
                    for kt in range(N // P):
                        sc = ps_sc.tile([P, 2, 512], F32)
                        for hs in range(2):
                            qo = hs * HD
                            nc.tensor.matmul(
                                sc[:, hs, :],
                                qkT_t[qo:qo + HD, KT + p, kt * P:(kt + 1) * P],
                                qkT_t[qo:qo + HD, p, hf * 512:(hf + 1) * 512],
                                start=True, stop=True,
                            )
                        ex = exp_p.tile([P, 2, 512], BF16)
                        nc.scalar.activation(ex[:], sc[:], EXP, scale=SCALE)
                        for hs in range(2):
                            nc.tensor.matmul(
                                po[hs][:],
                                v_t[:, kt, 2 * p + hs, :],
                                ex[:, hs, :],
                                start=(kt == 0), stop=(kt == N // P - 1),
                            )
                    for hs in range(2):
                        h = 2 * p + hs
                        rec = smallp.tile([1, 512], F32, tag="rec")
                        nc.vector.reciprocal(rec[:], po[hs][HD:HD + 1, :])
                        rbc = smallp.tile([HD, 512], F32, tag="rbc")
                        nc.gpsimd.partition_broadcast(rbc[:], rec[:])
                        nc.vector.tensor_tensor(
                            ao_t[hs * HD:(hs + 1) * HD, p, hf * 512:(hf + 1) * 512],
                            po[hs][0:HD, :], rbc[:], mybir.AluOpType.mult,
                        )

            # ---- output projection + bias ----
            for rt in range(N // P):
                y_t = yp.tile([P, D], F32)
                for j in range(2):
                    pm = ps_mm.tile([P, 512], F32)
                    for kt in range(KT):
                        nc.tensor.matmul(
                            pm[:, :384],
                            ao_t[:, kt, rt * P:(rt + 1) * P],
                            wproj_sb[:, kt, j * 384:(j + 1) * 384],
                            start=(kt == 0), stop=(kt == KT - 1),
                        )
                    nc.vector.tensor_add(
                        y_t[:, j * 384:(j + 1) * 384], pm[:, :384],
                        bias_sb[:, j * 384:(j + 1) * 384],
                    )
                nc.sync.dma_start(out[rows0 + rt * P:rows0 + (rt + 1) * P, :], y_t[:])

    nc.compile()
    return nc


_NC_CACHE = None


def _get_nc():
    global _NC_CACHE
    if _NC_CACHE is None:
        _NC_CACHE = build_kernel()
    return _NC_CACHE


def make_in_maps(x, W_qkv, W_proj, b_proj):
    x = np.asarray(x, np.float32)
    wq = np.asarray(W_qkv, np.float32).astype(ml_dtypes.bfloat16)
    wp = np.asarray(W_proj, np.float32).astype(ml_dtypes.bfloat16)
    bias = np.ascontiguousarray(
        np.broadcast_to(np.asarray(b_proj, np.float32), (P, D))
    )
    in_maps = []
    for c in range(NCORES):
        xc = x[BPC * c:BPC * (c + 1)].reshape(ROWS, D).T
        in_maps.append({
            "xT": np.ascontiguousarray(xc).astype(ml_dtypes.bfloat16),
            "wqkv": wq, "wproj": wp, "bias": bias,
        })
    return in_maps


def run(x, W_qkv, W_proj, b_proj, trace=False):
    nc = _get_nc()
    in_maps = make_in_maps(x, W_qkv, W_proj, b_proj)
    res = run_bass_kernel_spmd(nc, in_maps, core_ids=list(range(NCORES)), trace=trace)
    y = np.concatenate(
        [res.results[c]["out"].reshape(BPC, N, D) for c in range(NCORES)], axis=0
    )
    return y.astype(np.float32), res


def kernel(x, W_qkv, W_proj, b_proj):
    y, _ = run(x, W_qkv, W_proj, b_proj, trace=False)
    return y


# revision 5
# speedup vs baseline: 1.6718x; 1.6718x over previous
"""Multi-head attention block (B=16, N=1024, D=768, H=12) on 8 TRN2 NeuronCores.

Strategy: pure data parallelism — 2 batch items per core, no collectives.
Host pre-transposes x to x^T and casts operands to bf16, so the device
needs no transposes at all:
  - QKV for q,k computed in transposed orientation (qkT [1536, rows]):
    lhsT = W_qkv tile, rhs = x^T tile.
  - v computed in natural orientation [rows, 768] (PV lhsT wants it):
    lhsT = x^T tile, rhs = W_qkv v-columns; a ones column is appended
    per head so the PV matmul also produces the softmax denominators.
  - scores computed transposed [keys, rows] (softmax'd probabilities are
    then directly the PV moving operand). K=64 per head; heads are
    processed in pairs at base partitions 0/64 so the two matmuls pack
    into distinct PE row groups and run concurrently.
  - exp on ScalarE with the 1/sqrt(hd) scale fused; no max subtraction
    (scores are ~N(0,1) by construction, exp cannot overflow).
  - attention output accumulates as attn_out^T [768, rows], which is
    exactly the lhsT layout the output projection needs.
  - softmax normalization: reciprocal of the ones-row sums, gpsimd
    partition-broadcast, fused into the PSUM->SBUF copyback multiply.
"""

import sys
import types
import numpy as np
import ml_dtypes
from contextlib import ExitStack

# --- shim: provide antenv.axon_hooks so trace=True works under axon ---
if "antenv.axon_hooks" not in sys.modules:
    try:
        from trn_agent_boot.trn_boot import _ntff_profile_via_ctypes

        _hooks_mod = types.ModuleType("antenv.axon_hooks")
        _ntff_hook = _ntff_profile_via_ctypes("/opt/axon/libaxon_pjrt.so")
        _hooks_mod.get_axon_ntff_profile_hook = lambda: _ntff_hook
        _hooks_mod.set_axon_ntff_profile_hook = lambda h: None
        sys.modules["antenv.axon_hooks"] = _hooks_mod
    except Exception:
        pass

import concourse.bass as bass
import concourse.tile as tile
from concourse import bacc, mybir
import concourse.bass_utils as bass_utils
from concourse.bass_utils import run_bass_kernel_spmd

bass_utils.upload_artifacts = lambda tmpdir: tmpdir  # no S3 in sandbox

F32 = mybir.dt.float32
BF16 = mybir.dt.bfloat16
EXP = mybir.ActivationFunctionType.Exp

NCORES = 8
B, N, D = 16, 1024, 768
H, HD = 12, 64
BPC = B // NCORES        # batch items per core
ROWS = BPC * N           # 2048
P = 128
KT = D // P              # 6 contraction tiles
SCALE = HD ** -0.5


def build_kernel():
    nc = bacc.Bacc("TRN2", target_bir_lowering=False, debug=False, num_devices=NCORES)
    xT = nc.dram_tensor("xT", [D, ROWS], BF16, kind="ExternalInput").ap()
    wqkv = nc.dram_tensor("wqkv", [D, 3 * D], BF16, kind="ExternalInput").ap()
    wproj = nc.dram_tensor("wproj", [D, D], BF16, kind="ExternalInput").ap()
    bias = nc.dram_tensor("bias", [P, D], F32, kind="ExternalInput").ap()
    out = nc.dram_tensor("out", [ROWS, D], F32, kind="ExternalOutput").ap()

    with tile.TileContext(nc) as tc, ExitStack() as ctx:
        const = ctx.enter_context(tc.tile_pool(name="const", bufs=1))
        xp = ctx.enter_context(tc.tile_pool(name="xT", bufs=2))
        qkp = ctx.enter_context(tc.tile_pool(name="qkT", bufs=2))
        vp = ctx.enter_context(tc.tile_pool(name="v", bufs=2))
        aop = ctx.enter_context(tc.tile_pool(name="ao", bufs=2))
        exp_p = ctx.enter_context(tc.tile_pool(name="exp", bufs=3))
        yp = ctx.enter_context(tc.tile_pool(name="y", bufs=3))
        smallp = ctx.enter_context(tc.tile_pool(name="small", bufs=3))
        ps_mm = ctx.enter_context(tc.tile_pool(name="ps_mm", bufs=2, space="PSUM"))
        ps_sc = ctx.enter_context(tc.tile_pool(name="ps_sc", bufs=2, space="PSUM"))
        ps_out = ctx.enter_context(tc.tile_pool(name="ps_out", bufs=2, space="PSUM"))

        # resident weights
        wqkv_sb = const.tile([P, KT, 3 * D], BF16)
        nc.sync.dma_start(wqkv_sb[:], wqkv.rearrange("(a p) n -> p a n", p=P))
        wproj_sb = const.tile([P, KT, D], BF16)
        nc.sync.dma_start(wproj_sb[:], wproj.rearrange("(a p) n -> p a n", p=P))
        bias_sb = const.tile([P, D], F32)
        nc.sync.dma_start(bias_sb[:], bias)

        for b in range(BPC):
            rows0 = b * N
            xT_t = xp.tile([P, KT, N], BF16)
            nc.sync.dma_start(
                xT_t[:], xT[:, rows0:rows0 + N].rearrange("(a p) n -> p a n", p=P)
            )

            # ---- QKV: q,k transposed [1536, N] ----
            qkT_t = qkp.tile([P, 2 * KT, N], BF16)
            for nt in range(2 * KT):
                for hf in range(2):
                    pm = ps_mm.tile([P, 512], F32)
                    for kt in range(KT):
                        nc.tensor.matmul(
                            pm[:],
                            wqkv_sb[:, kt, nt * P:(nt + 1) * P],
                            xT_t[:, kt, hf * 512:(hf + 1) * 512],
                            start=(kt == 0), stop=(kt == KT - 1),
                        )
                    nc.vector.tensor_copy(qkT_t[:, nt, hf * 512:(hf + 1) * 512], pm[:])

            # ---- V natural [N, 768] with appended ones column per head ----
            v_t = vp.tile([P, N // P, H, HD + 1], BF16)
            nc.vector.memset(v_t[:, :, :, HD:HD + 1], 1.0)
            for rt in range(N // P):
                for j in range(2):
                    pm = ps_mm.tile([P, 512], F32)
                    for kt in range(KT):
                        nc.tensor.matmul(
                            pm[:, :384],
                            xT_t[:, kt, rt * P:(rt + 1) * P],
                            wqkv_sb[:, kt, 2 * D + j * 384:2 * D + (j + 1) * 384],
                            start=(kt == 0), stop=(kt == KT - 1),
                        )
                    nc.vector.tensor_copy(v_t[:, rt, j * 6:(j + 1) * 6, 0:HD], pm[:, :384])

            # ---- attention, head pairs, transposed scores ----
            ao_t = aop.tile([P, KT, N], BF16)
            for p in range(H // 2):
                for hf in range(2):
                    po = [
                        ps_out.tile([HD + 1, 512], F32, tag="po", name=f"po_{b}_{p}_{hf}_{hs}")
                        for hs in range(2)
                    ]
                    for kt in range(N // P):
                        sc = ps_sc.tile([P, 2, 512], F32)
                        for hs in range(2):
                            qo = hs * HD
                            nc.tensor.matmul(
                                sc[:, hs, :],
                                qkT_t[qo:qo + HD, KT + p, kt * P:(kt + 1) * P],
                                qkT_t[qo:qo + HD, p, hf * 512:(hf + 1) * 512],
                                start=True, stop=True,
                            )
                        ex = exp_p.tile([P, 2, 512], BF16)
                        nc.scalar.activation(ex[:], sc[:], EXP, scale=SCALE)
                        for hs in range(2):
                            nc.tensor.matmul(
                                po[hs][:],
                                v_t[:, kt, 2 * p + hs, :],
                                ex[:, hs, :],
                                start=(kt == 0), stop=(kt == N // P - 1),
                            )
                    for hs in range(2):
                        h = 2 * p + hs
                        # two fast copies release the PSUM bank; normalization
                        # happens off the critical path in SBUF.
                        # (partition_broadcast always reads partition 0, so the
                        # sums row gets its own base-0 tile)
                        u_t = smallp.tile([HD, 512], F32, tag="u")
                        nc.vector.tensor_copy(u_t[:], po[hs][0:HD, :])
                        sums_t = smallp.tile([1, 512], F32, tag="sums")
                        nc.vector.tensor_copy(sums_t[:], po[hs][HD:HD + 1, :])
                        rbc = smallp.tile([HD, 512], F32, tag="rbc")
                        nc.gpsimd.partition_broadcast(rbc[:], sums_t[:])
                        rec = smallp.tile([HD, 512], F32, tag="rec")
                        nc.vector.reciprocal_approx_fast(rec[:], rbc[:])
                        nc.vector.tensor_tensor(
                            ao_t[hs * HD:(hs + 1) * HD, p, hf * 512:(hf + 1) * 512],
                            u_t[:], rec[:], mybir.AluOpType.mult,
                        )

            # ---- output projection + bias ----
            for rt in range(N // P):
                y_t = yp.tile([P, D], F32)
                for j in range(2):
                    pm = ps_mm.tile([P, 512], F32)
                    for kt in range(KT):
                        nc.tensor.matmul(
                            pm[:, :384],
                            ao_t[:, kt, rt * P:(rt + 1) * P],
                            wproj_sb[:, kt, j * 384:(j + 1) * 384],
                            start=(kt == 0), stop=(kt == KT - 1),
                        )
                    nc.vector.tensor_add(
                        y_t[:, j * 384:(j + 1) * 384], pm[:, :384],
                        bias_sb[:, j * 384:(j + 1) * 384],
                    )
                nc.sync.dma_start(out[rows0 + rt * P:rows0 + (rt + 1) * P, :], y_t[:])

    nc.compile()
    return nc


_NC_CACHE = None


def _get_nc():
    global _NC_CACHE
    if _NC_CACHE is None:
        _NC_CACHE = build_kernel()
    return _NC_CACHE


def make_in_maps(x, W_qkv, W_proj, b_proj):
    x = np.asarray(x, np.float32)
    wq = np.asarray(W_qkv, np.float32).astype(ml_dtypes.bfloat16)
    wp = np.asarray(W_proj, np.float32).astype(ml_dtypes.bfloat16)
    bias = np.ascontiguousarray(
        np.broadcast_to(np.asarray(b_proj, np.float32), (P, D))
    )
    in_maps = []
    for c in range(NCORES):
        xc = x[BPC * c:BPC * (c + 1)].reshape(ROWS, D).T
        in_maps.append({
            "xT": np.ascontiguousarray(xc).astype(ml_dtypes.bfloat16),
            "wqkv": wq, "wproj": wp, "bias": bias,
        })
    return in_maps


def run(x, W_qkv, W_proj, b_proj, trace=False):
    nc = _get_nc()
    in_maps = make_in_maps(x, W_qkv, W_proj, b_proj)
    res = run_bass_kernel_spmd(nc, in_maps, core_ids=list(range(NCORES)), trace=trace)
    y = np.concatenate(
        [res.results[c]["out"].reshape(BPC, N, D) for c in range(NCORES)], axis=0
    )
    return y.astype(np.float32), res


def kernel(x, W_qkv, W_proj, b_proj):
    y, _ = run(x, W_qkv, W_proj, b_proj, trace=False)
    return y


# revision 6
# speedup vs baseline: 1.8757x; 1.1220x over previous
"""Multi-head attention block (B=16, N=1024, D=768, H=12) on 8 TRN2 NeuronCores.

Strategy: pure data parallelism — 2 batch items per core, no collectives.
Host pre-transposes x to x^T and casts operands to bf16, so the device
needs no transposes at all:
  - QKV for q,k computed in transposed orientation (qkT [1536, rows]):
    lhsT = W_qkv tile, rhs = x^T tile.
  - v computed in natural orientation [rows, 768] (PV lhsT wants it):
    lhsT = x^T tile, rhs = W_qkv v-columns; a ones column is appended
    per head so the PV matmul also produces the softmax denominators.
  - scores computed transposed [keys, rows] (softmax'd probabilities are
    then directly the PV moving operand). K=64 per head; heads are
    processed in pairs at base partitions 0/64 so the two matmuls pack
    into distinct PE row groups and run concurrently.
  - exp on ScalarE with the 1/sqrt(hd) scale fused; no max subtraction
    (scores are ~N(0,1) by construction, exp cannot overflow).
  - attention output accumulates as attn_out^T [768, rows], which is
    exactly the lhsT layout the output projection needs.
  - softmax normalization: reciprocal of the ones-row sums, gpsimd
    partition-broadcast, fused into the PSUM->SBUF copyback multiply.
"""

import sys
import types
import numpy as np
import ml_dtypes
from contextlib import ExitStack

# --- shim: provide antenv.axon_hooks so trace=True works under axon ---
if "antenv.axon_hooks" not in sys.modules:
    try:
        from trn_agent_boot.trn_boot import _ntff_profile_via_ctypes

        _hooks_mod = types.ModuleType("antenv.axon_hooks")
        _ntff_hook = _ntff_profile_via_ctypes("/opt/axon/libaxon_pjrt.so")
        _hooks_mod.get_axon_ntff_profile_hook = lambda: _ntff_hook
        _hooks_mod.set_axon_ntff_profile_hook = lambda h: None
        sys.modules["antenv.axon_hooks"] = _hooks_mod
    except Exception:
        pass

import concourse.bass as bass
import concourse.tile as tile
from concourse import bacc, mybir
import concourse.bass_utils as bass_utils
from concourse.bass_utils import run_bass_kernel_spmd

bass_utils.upload_artifacts = lambda tmpdir: tmpdir  # no S3 in sandbox

F32 = mybir.dt.float32
BF16 = mybir.dt.bfloat16
EXP = mybir.ActivationFunctionType.Exp

NCORES = 8
B, N, D = 16, 1024, 768
H, HD = 12, 64
BPC = B // NCORES        # batch items per core
ROWS = BPC * N           # 2048
P = 128
KT = D // P              # 6 contraction tiles
SCALE = HD ** -0.5


def build_kernel():
    nc = bacc.Bacc("TRN2", target_bir_lowering=False, debug=False, num_devices=NCORES)
    xT = nc.dram_tensor("xT", [D, ROWS], BF16, kind="ExternalInput").ap()
    wqkv = nc.dram_tensor("wqkv", [D, 3 * D], BF16, kind="ExternalInput").ap()
    wproj = nc.dram_tensor("wproj", [D, D], BF16, kind="ExternalInput").ap()
    bias = nc.dram_tensor("bias", [P, D], F32, kind="ExternalInput").ap()
    out = nc.dram_tensor("out", [ROWS, D], F32, kind="ExternalOutput").ap()

    with tile.TileContext(nc) as tc, ExitStack() as ctx:
        const = ctx.enter_context(tc.tile_pool(name="const", bufs=1))
        xp = ctx.enter_context(tc.tile_pool(name="xT", bufs=2))
        qkp = ctx.enter_context(tc.tile_pool(name="qkT", bufs=2))
        vp = ctx.enter_context(tc.tile_pool(name="v", bufs=2))
        aop = ctx.enter_context(tc.tile_pool(name="ao", bufs=2))
        exp_p = ctx.enter_context(tc.tile_pool(name="exp", bufs=3))
        yp = ctx.enter_context(tc.tile_pool(name="y", bufs=3))
        smallp = ctx.enter_context(tc.tile_pool(name="small", bufs=3))
        ps_mm = ctx.enter_context(tc.tile_pool(name="ps_mm", bufs=2, space="PSUM"))
        ps_sc = ctx.enter_context(tc.tile_pool(name="ps_sc", bufs=2, space="PSUM"))
        ps_out = ctx.enter_context(tc.tile_pool(name="ps_out", bufs=2, space="PSUM"))

        # resident weights
        wqkv_sb = const.tile([P, KT, 3 * D], BF16)
        nc.sync.dma_start(wqkv_sb[:], wqkv.rearrange("(a p) n -> p a n", p=P))
        wproj_sb = const.tile([P, KT, D], BF16)
        nc.sync.dma_start(wproj_sb[:], wproj.rearrange("(a p) n -> p a n", p=P))
        bias_sb = const.tile([P, D], F32)
        nc.sync.dma_start(bias_sb[:], bias)

        for b in range(BPC):
            rows0 = b * N
            xT_t = xp.tile([P, KT, N], BF16)
            nc.sync.dma_start(
                xT_t[:], xT[:, rows0:rows0 + N].rearrange("(a p) n -> p a n", p=P)
            )

            # ---- V natural [N, 768] with appended ones column per head ----
            v_t = vp.tile([P, N // P, H, HD + 1], BF16)
            nc.vector.memset(v_t[:, :, :, HD:HD + 1], 1.0)
            for rt in range(N // P):
                for j in range(2):
                    pm = ps_mm.tile([P, 512], F32)
                    for kt in range(KT):
                        nc.tensor.matmul(
                            pm[:, :384],
                            xT_t[:, kt, rt * P:(rt + 1) * P],
                            wqkv_sb[:, kt, 2 * D + j * 384:2 * D + (j + 1) * 384],
                            start=(kt == 0), stop=(kt == KT - 1),
                        )
                    nc.vector.tensor_copy(v_t[:, rt, j * 6:(j + 1) * 6, 0:HD], pm[:, :384])

            # ---- per head pair: qk projection for its two tiles, then attention ----
            qkT_t = qkp.tile([P, 2 * KT, N], BF16)
            ao_t = aop.tile([P, KT, N], BF16)
            for p in range(H // 2):
                for nt in (p, KT + p):
                    for hf in range(2):
                        pm = ps_mm.tile([P, 512], F32)
                        for kt in range(KT):
                            nc.tensor.matmul(
                                pm[:],
                                wqkv_sb[:, kt, nt * P:(nt + 1) * P],
                                xT_t[:, kt, hf * 512:(hf + 1) * 512],
                                start=(kt == 0), stop=(kt == KT - 1),
                            )
                        nc.vector.tensor_copy(qkT_t[:, nt, hf * 512:(hf + 1) * 512], pm[:])

                for hf in range(2):
                    po = [
                        ps_out.tile([HD + 1, 512], F32, tag="po", name=f"po_{b}_{p}_{hf}_{hs}")
                        for hs in range(2)
                    ]
                    for kt in range(N // P):
                        sc = ps_sc.tile([P, 2, 512], F32)
                        for hs in range(2):
                            qo = hs * HD
                            nc.tensor.matmul(
                                sc[:, hs, :],
                                qkT_t[qo:qo + HD, KT + p, kt * P:(kt + 1) * P],
                                qkT_t[qo:qo + HD, p, hf * 512:(hf + 1) * 512],
                                start=True, stop=True,
                            )
                        ex = exp_p.tile([P, 2, 512], BF16)
                        nc.scalar.activation(ex[:], sc[:], EXP, scale=SCALE)
                        for hs in range(2):
                            nc.tensor.matmul(
                                po[hs][:],
                                v_t[:, kt, 2 * p + hs, :],
                                ex[:, hs, :],
                                start=(kt == 0), stop=(kt == N // P - 1),
                            )
                    for hs in range(2):
                        h = 2 * p + hs
                        # two fast copies release the PSUM bank; normalization
                        # happens off the critical path in SBUF.
                        # (partition_broadcast always reads partition 0, so the
                        # sums row gets its own base-0 tile)
                        u_t = smallp.tile([HD, 512], F32, tag="u")
                        nc.vector.tensor_copy(u_t[:], po[hs][0:HD, :])
                        sums_t = smallp.tile([1, 512], F32, tag="sums")
                        nc.vector.tensor_copy(sums_t[:], po[hs][HD:HD + 1, :])
                        rbc = smallp.tile([HD, 512], F32, tag="rbc")
                        nc.gpsimd.partition_broadcast(rbc[:], sums_t[:])
                        rec = smallp.tile([HD, 512], F32, tag="rec")
                        nc.vector.reciprocal_approx_fast(rec[:], rbc[:])
                        nc.vector.tensor_tensor(
                            ao_t[hs * HD:(hs + 1) * HD, p, hf * 512:(hf + 1) * 512],
                            u_t[:], rec[:], mybir.AluOpType.mult,
                        )

            # ---- output projection + bias ----
            for rt in range(N // P):
                y_t = yp.tile([P, D], F32)
                for j in range(2):
                    pm = ps_mm.tile([P, 512], F32)
                    for kt in range(KT):
                        nc.tensor.matmul(
                            pm[:, :384],
                            ao_t[:, kt, rt * P:(rt + 1) * P],
                            wproj_sb[:, kt, j * 384:(j + 1) * 384],
                            start=(kt == 0), stop=(kt == KT - 1),
                        )
                    nc.vector.tensor_add(
                        y_t[:, j * 384:(j + 1) * 384], pm[:, :384],
                        bias_sb[:, j * 384:(j + 1) * 384],
                    )
                nc.sync.dma_start(out[rows0 + rt * P:rows0 + (rt + 1) * P, :], y_t[:])

    nc.compile()
    return nc


_NC_CACHE = None


def _get_nc():
    global _NC_CACHE
    if _NC_CACHE is None:
        _NC_CACHE = build_kernel()
    return _NC_CACHE


def make_in_maps(x, W_qkv, W_proj, b_proj):
    x = np.asarray(x, np.float32)
    wq = np.asarray(W_qkv, np.float32).astype(ml_dtypes.bfloat16)
    wp = np.asarray(W_proj, np.float32).astype(ml_dtypes.bfloat16)
    bias = np.ascontiguousarray(
        np.broadcast_to(np.asarray(b_proj, np.float32), (P, D))
    )
    in_maps = []
    for c in range(NCORES):
        xc = x[BPC * c:BPC * (c + 1)].reshape(ROWS, D).T
        in_maps.append({
            "xT": np.ascontiguousarray(xc).astype(ml_dtypes.bfloat16),
            "wqkv": wq, "wproj": wp, "bias": bias,
        })
    return in_maps


def run(x, W_qkv, W_proj, b_proj, trace=False):
    nc = _get_nc()
    in_maps = make_in_maps(x, W_qkv, W_proj, b_proj)
    res = run_bass_kernel_spmd(nc, in_maps, core_ids=list(range(NCORES)), trace=trace)
    y = np.concatenate(
        [res.results[c]["out"].reshape(BPC, N, D) for c in range(NCORES)], axis=0
    )
    return y.astype(np.float32), res


def kernel(x, W_qkv, W_proj, b_proj):
    y, _ = run(x, W_qkv, W_proj, b_proj, trace=False)
    return y


# revision 8
# speedup vs baseline: 1.9366x; 1.0325x over previous
"""Multi-head attention block (B=16, N=1024, D=768, H=12) on 8 TRN2 NeuronCores.

Strategy: pure data parallelism — 2 batch items per core, no collectives.
Host pre-transposes x to x^T and casts operands to bf16, so the device
needs no transposes at all:
  - QKV for q,k computed in transposed orientation (qkT [1536, rows]):
    lhsT = W_qkv tile, rhs = x^T tile.
  - v computed in natural orientation [rows, 768] (PV lhsT wants it):
    lhsT = x^T tile, rhs = W_qkv v-columns; a ones column is appended
    per head so the PV matmul also produces the softmax denominators.
  - scores computed transposed [keys, rows] (softmax'd probabilities are
    then directly the PV moving operand). K=64 per head; heads are
    processed in pairs at base partitions 0/64 so the two matmuls pack
    into distinct PE row groups and run concurrently.
  - exp on ScalarE with the 1/sqrt(hd) scale fused; no max subtraction
    (scores are ~N(0,1) by construction, exp cannot overflow).
  - attention output accumulates as attn_out^T [768, rows], which is
    exactly the lhsT layout the output projection needs.
  - softmax normalization: reciprocal of the ones-row sums, gpsimd
    partition-broadcast, fused into the PSUM->SBUF copyback multiply.
"""

import sys
import types
import numpy as np
import ml_dtypes
from contextlib import ExitStack

# --- shim: provide antenv.axon_hooks so trace=True works under axon ---
if "antenv.axon_hooks" not in sys.modules:
    try:
        from trn_agent_boot.trn_boot import _ntff_profile_via_ctypes

        _hooks_mod = types.ModuleType("antenv.axon_hooks")
        _ntff_hook = _ntff_profile_via_ctypes("/opt/axon/libaxon_pjrt.so")
        _hooks_mod.get_axon_ntff_profile_hook = lambda: _ntff_hook
        _hooks_mod.set_axon_ntff_profile_hook = lambda h: None
        sys.modules["antenv.axon_hooks"] = _hooks_mod
    except Exception:
        pass

import concourse.bass as bass
import concourse.tile as tile
from concourse import bacc, mybir
import concourse.bass_utils as bass_utils
from concourse.bass_utils import run_bass_kernel_spmd

bass_utils.upload_artifacts = lambda tmpdir: tmpdir  # no S3 in sandbox

F32 = mybir.dt.float32
BF16 = mybir.dt.bfloat16
EXP = mybir.ActivationFunctionType.Exp

NCORES = 8
B, N, D = 16, 1024, 768
H, HD = 12, 64
BPC = B // NCORES        # batch items per core
ROWS = BPC * N           # 2048
P = 128
KT = D // P              # 6 contraction tiles
SCALE = HD ** -0.5


def build_kernel():
    nc = bacc.Bacc("TRN2", target_bir_lowering=False, debug=False, num_devices=NCORES)
    xT = nc.dram_tensor("xT", [D, ROWS], BF16, kind="ExternalInput").ap()
    wqkv = nc.dram_tensor("wqkv", [D, 3 * D], BF16, kind="ExternalInput").ap()
    wproj = nc.dram_tensor("wproj", [D, D], BF16, kind="ExternalInput").ap()
    bias = nc.dram_tensor("bias", [P, D], F32, kind="ExternalInput").ap()
    out = nc.dram_tensor("out", [ROWS, D], F32, kind="ExternalOutput").ap()

    with tile.TileContext(nc) as tc, ExitStack() as ctx:
        const = ctx.enter_context(tc.tile_pool(name="const", bufs=1))
        xp = ctx.enter_context(tc.tile_pool(name="xT", bufs=2))
        qkp = ctx.enter_context(tc.tile_pool(name="qkT", bufs=2))
        vp = ctx.enter_context(tc.tile_pool(name="v", bufs=2))
        aop = ctx.enter_context(tc.tile_pool(name="ao", bufs=1))
        exp_p = ctx.enter_context(tc.tile_pool(name="exp", bufs=3))
        yp = ctx.enter_context(tc.tile_pool(name="y", bufs=2))
        smallp = ctx.enter_context(tc.tile_pool(name="small", bufs=2))
        ps_mm = ctx.enter_context(tc.tile_pool(name="ps_mm", bufs=2, space="PSUM"))
        ps_sc = ctx.enter_context(tc.tile_pool(name="ps_sc", bufs=2, space="PSUM"))
        ps_out = ctx.enter_context(tc.tile_pool(name="ps_out", bufs=2, space="PSUM"))

        # resident weights — v columns first so the v phase can start ASAP
        wv_sb = const.tile([P, KT, D], BF16)
        nc.sync.dma_start(wv_sb[:], wqkv[:, 2 * D:3 * D].rearrange("(a p) n -> p a n", p=P))
        # prefetch both batches' activations up front
        xT_ts = []
        for b in range(BPC):
            xT_t = xp.tile([P, KT, N], BF16, name=f"xT_{b}")
            nc.sync.dma_start(
                xT_t[:], xT[:, b * N:(b + 1) * N].rearrange("(a p) n -> p a n", p=P)
            )
            xT_ts.append(xT_t)
        wqk_sb = const.tile([P, KT, 2 * D], BF16)
        nc.sync.dma_start(wqk_sb[:], wqkv[:, 0:2 * D].rearrange("(a p) n -> p a n", p=P))
        wproj_sb = const.tile([P, KT, D], BF16)
        nc.sync.dma_start(wproj_sb[:], wproj.rearrange("(a p) n -> p a n", p=P))
        bias_sb = const.tile([P, D], F32)
        nc.sync.dma_start(bias_sb[:], bias)

        for b in range(BPC):
            rows0 = b * N
            xT_t = xT_ts[b]

            # ---- V natural [N, 768] with appended ones column per head ----
            v_t = vp.tile([P, N // P, H, HD + 1], BF16)
            nc.vector.memset(v_t[:, :, :, HD:HD + 1], 1.0)
            for rt in range(N // P):
                for j in range(2):
                    pm = ps_mm.tile([P, 512], F32)
                    for kt in range(KT):
                        nc.tensor.matmul(
                            pm[:, :384],
                            xT_t[:, kt, rt * P:(rt + 1) * P],
                            wv_sb[:, kt, j * 384:(j + 1) * 384],
                            start=(kt == 0), stop=(kt == KT - 1),
                        )
                    nc.vector.tensor_copy(v_t[:, rt, j * 6:(j + 1) * 6, 0:HD], pm[:, :384])

            # ---- per head pair: qk projection for its two tiles, then attention ----
            qkT_t = qkp.tile([P, 2 * KT, N], BF16)
            ao_t = aop.tile([P, KT, N], BF16)
            for p in range(H // 2):
                for nt in (p, KT + p):
                    for hf in range(2):
                        pm = ps_mm.tile([P, 512], F32)
                        for kt in range(KT):
                            nc.tensor.matmul(
                                pm[:],
                                wqk_sb[:, kt, nt * P:(nt + 1) * P],
                                xT_t[:, kt, hf * 512:(hf + 1) * 512],
                                start=(kt == 0), stop=(kt == KT - 1),
                            )
                        nc.vector.tensor_copy(qkT_t[:, nt, hf * 512:(hf + 1) * 512], pm[:])

                for hf in range(2):
                    po = [
                        ps_out.tile([HD + 1, 512], F32, tag="po", name=f"po_{b}_{p}_{hf}_{hs}")
                        for hs in range(2)
                    ]
                    for kt in range(N // P):
                        sc = ps_sc.tile([P, 2, 512], F32)
                        for hs in range(2):
                            qo = hs * HD
                            nc.tensor.matmul(
                                sc[:, hs, :],
                                qkT_t[qo:qo + HD, KT + p, kt * P:(kt + 1) * P],
                                qkT_t[qo:qo + HD, p, hf * 512:(hf + 1) * 512],
                                start=True, stop=True,
                            )
                        ex = exp_p.tile([P, 2, 512], BF16)
                        nc.scalar.activation(ex[:], sc[:], EXP, scale=SCALE)
                        for hs in range(2):
                            nc.tensor.matmul(
                                po[hs][:],
                                v_t[:, kt, 2 * p + hs, :],
                                ex[:, hs, :],
                                start=(kt == 0), stop=(kt == N // P - 1),
                            )
                    for hs in range(2):
                        h = 2 * p + hs
                        # two fast copies release the PSUM bank; normalization
                        # happens off the critical path in SBUF.
                        # (partition_broadcast always reads partition 0, so the
                        # sums row gets its own base-0 tile)
                        u_t = smallp.tile([HD, 512], F32, tag="u")
                        nc.vector.tensor_copy(u_t[:], po[hs][0:HD, :])
                        sums_t = smallp.tile([1, 512], F32, tag="sums")
                        nc.vector.tensor_copy(sums_t[:], po[hs][HD:HD + 1, :])
                        rbc = smallp.tile([HD, 512], F32, tag="rbc")
                        nc.gpsimd.partition_broadcast(rbc[:], sums_t[:])
                        rec = smallp.tile([HD, 512], F32, tag="rec")
                        nc.vector.reciprocal_approx_fast(rec[:], rbc[:])
                        nc.vector.tensor_tensor(
                            ao_t[hs * HD:(hs + 1) * HD, p, hf * 512:(hf + 1) * 512],
                            u_t[:], rec[:], mybir.AluOpType.mult,
                        )

            # ---- output projection + bias ----
            for rt in range(N // P):
                y_t = yp.tile([P, D], F32)
                for j in range(2):
                    pm = ps_mm.tile([P, 512], F32)
                    for kt in range(KT):
                        nc.tensor.matmul(
                            pm[:, :384],
                            ao_t[:, kt, rt * P:(rt + 1) * P],
                            wproj_sb[:, kt, j * 384:(j + 1) * 384],
                            start=(kt == 0), stop=(kt == KT - 1),
                        )
                    nc.vector.tensor_add(
                        y_t[:, j * 384:(j + 1) * 384], pm[:, :384],
                        bias_sb[:, j * 384:(j + 1) * 384],
                    )
                nc.sync.dma_start(out[rows0 + rt * P:rows0 + (rt + 1) * P, :], y_t[:])

    nc.compile()
    return nc


_NC_CACHE = None


def _get_nc():
    global _NC_CACHE
    if _NC_CACHE is None:
        _NC_CACHE = build_kernel()
    return _NC_CACHE


def make_in_maps(x, W_qkv, W_proj, b_proj):
    x = np.asarray(x, np.float32)
    wq = np.asarray(W_qkv, np.float32).astype(ml_dtypes.bfloat16)
    wp = np.asarray(W_proj, np.float32).astype(ml_dtypes.bfloat16)
    bias = np.ascontiguousarray(
        np.broadcast_to(np.asarray(b_proj, np.float32), (P, D))
    )
    in_maps = []
    for c in range(NCORES):
        xc = x[BPC * c:BPC * (c + 1)].reshape(ROWS, D).T
        in_maps.append({
            "xT": np.ascontiguousarray(xc).astype(ml_dtypes.bfloat16),
            "wqkv": wq, "wproj": wp, "bias": bias,
        })
    return in_maps


def run(x, W_qkv, W_proj, b_proj, trace=False):
    nc = _get_nc()
    in_maps = make_in_maps(x, W_qkv, W_proj, b_proj)
    res = run_bass_kernel_spmd(nc, in_maps, core_ids=list(range(NCORES)), trace=trace)
    y = np.concatenate(
        [res.results[c]["out"].reshape(BPC, N, D) for c in range(NCORES)], axis=0
    )
    return y.astype(np.float32), res


def kernel(x, W_qkv, W_proj, b_proj):
    y, _ = run(x, W_qkv, W_proj, b_proj, trace=False)
    return y


# revision 12
# speedup vs baseline: 1.9770x; 1.0208x over previous
"""Multi-head attention block (B=16, N=1024, D=768, H=12) on 8 TRN2 NeuronCores.

Strategy: pure data parallelism — 2 batch items per core, no collectives.
Host pre-transposes x to x^T and casts operands to bf16, so the device
needs no transposes at all:
  - QKV for q,k computed in transposed orientation (qkT [1536, rows]):
    lhsT = W_qkv tile, rhs = x^T tile.
  - v computed in natural orientation [rows, 768] (PV lhsT wants it):
    lhsT = x^T tile, rhs = W_qkv v-columns; a ones column is appended
    per head so the PV matmul also produces the softmax denominators.
  - scores computed transposed [keys, rows] (softmax'd probabilities are
    then directly the PV moving operand). K=64 per head; heads are
    processed in pairs at base partitions 0/64 so the two matmuls pack
    into distinct PE row groups and run concurrently.
  - exp on ScalarE with the 1/sqrt(hd) scale fused; no max subtraction
    (scores are ~N(0,1) by construction, exp cannot overflow).
  - attention output accumulates as attn_out^T [768, rows], which is
    exactly the lhsT layout the output projection needs.
  - softmax normalization: reciprocal of the ones-row sums, gpsimd
    partition-broadcast, fused into the PSUM->SBUF copyback multiply.
"""

import sys
import types
import numpy as np
import ml_dtypes
from contextlib import ExitStack

# --- shim: provide antenv.axon_hooks so trace=True works under axon ---
if "antenv.axon_hooks" not in sys.modules:
    try:
        from trn_agent_boot.trn_boot import _ntff_profile_via_ctypes

        _hooks_mod = types.ModuleType("antenv.axon_hooks")
        _ntff_hook = _ntff_profile_via_ctypes("/opt/axon/libaxon_pjrt.so")
        _hooks_mod.get_axon_ntff_profile_hook = lambda: _ntff_hook
        _hooks_mod.set_axon_ntff_profile_hook = lambda h: None
        sys.modules["antenv.axon_hooks"] = _hooks_mod
    except Exception:
        pass

import concourse.bass as bass
import concourse.tile as tile
from concourse import bacc, mybir
import concourse.bass_utils as bass_utils
from concourse.bass_utils import run_bass_kernel_spmd

bass_utils.upload_artifacts = lambda tmpdir: tmpdir  # no S3 in sandbox

F32 = mybir.dt.float32
BF16 = mybir.dt.bfloat16
EXP = mybir.ActivationFunctionType.Exp

NCORES = 8
B, N, D = 16, 1024, 768
H, HD = 12, 64
BPC = B // NCORES        # batch items per core
ROWS = BPC * N           # 2048
P = 128
KT = D // P              # 6 contraction tiles
SCALE = HD ** -0.5


def build_kernel():
    nc = bacc.Bacc("TRN2", target_bir_lowering=False, debug=False, num_devices=NCORES)
    xT = nc.dram_tensor("xT", [D, ROWS], BF16, kind="ExternalInput").ap()
    wqkv = nc.dram_tensor("wqkv", [D, 3 * D], BF16, kind="ExternalInput").ap()
    wproj = nc.dram_tensor("wproj", [D, D], BF16, kind="ExternalInput").ap()
    bias = nc.dram_tensor("bias", [P, D], F32, kind="ExternalInput").ap()
    out = nc.dram_tensor("out", [ROWS, D], F32, kind="ExternalOutput").ap()

    with tile.TileContext(nc) as tc, ExitStack() as ctx:
        const = ctx.enter_context(tc.tile_pool(name="const", bufs=1))
        xp = ctx.enter_context(tc.tile_pool(name="xT", bufs=2))
        qkp = ctx.enter_context(tc.tile_pool(name="qkT", bufs=2))
        vp = ctx.enter_context(tc.tile_pool(name="v", bufs=2))
        aop = ctx.enter_context(tc.tile_pool(name="ao", bufs=2))
        exp_p = ctx.enter_context(tc.tile_pool(name="exp", bufs=3))
        yp = ctx.enter_context(tc.tile_pool(name="y", bufs=2))
        smallp = ctx.enter_context(tc.tile_pool(name="small", bufs=2))
        ps_mm = ctx.enter_context(tc.tile_pool(name="ps_mm", bufs=2, space="PSUM"))
        ps_sc = ctx.enter_context(tc.tile_pool(name="ps_sc", bufs=2, space="PSUM"))
        ps_out = ctx.enter_context(tc.tile_pool(name="ps_out", bufs=2, space="PSUM"))

        # resident weights — v columns first so the v phase can start ASAP
        wv_sb = const.tile([P, KT, D], BF16)
        for kt in range(KT):
            nc.sync.dma_start(
                wv_sb[:, kt, :], wqkv[kt * P:(kt + 1) * P, 2 * D:3 * D]
            )
        # prefetch both batches' activations up front
        xT_ts = []
        for b in range(BPC):
            xT_t = xp.tile([P, KT, N], BF16, tag="xT", name=f"xT_{b}")
            for kt in range(KT):
                nc.sync.dma_start(
                    xT_t[:, kt, :], xT[kt * P:(kt + 1) * P, b * N:(b + 1) * N]
                )
            xT_ts.append(xT_t)
        wqk_sb = const.tile([P, KT, 2 * D], BF16)
        nc.sync.dma_start(wqk_sb[:], wqkv[:, 0:2 * D].rearrange("(a p) n -> p a n", p=P))
        wproj_sb = const.tile([P, KT, D], BF16)
        nc.sync.dma_start(wproj_sb[:], wproj.rearrange("(a p) n -> p a n", p=P))
        bias_sb = const.tile([P, D], F32)
        nc.sync.dma_start(bias_sb[:], bias)

        def v_phase(b):
            xT_t = xT_ts[b]
            v_flat = vp.tile([P, N // P, H * (HD + 1)], BF16, tag="v", name=f"v_{b}")
            v_t = v_flat[:].rearrange("q a (h c) -> q a h c", h=H)
            nc.vector.memset(v_t[:, :, :, HD:HD + 1], 1.0)
            for rt in range(N // P):
                for j in range(2):
                    pm = ps_mm.tile([P, 512], F32)
                    for kt in range(KT):
                        nc.tensor.matmul(
                            pm[:, :384],
                            xT_t[:, kt, rt * P:(rt + 1) * P],
                            wv_sb[:, kt, j * 384:(j + 1) * 384],
                            start=(kt == 0), stop=(kt == KT - 1),
                        )
                    nc.vector.tensor_copy(v_t[:, rt, j * 6:(j + 1) * 6, 0:HD], pm[:, :384])
            return v_t

        def qk_pair(b, qkT_t, p):
            xT_t = xT_ts[b]
            for nt in (p, KT + p):
                for hf in range(2):
                    pm = ps_mm.tile([P, 512], F32)
                    for kt in range(KT):
                        nc.tensor.matmul(
                            pm[:],
                            wqk_sb[:, kt, nt * P:(nt + 1) * P],
                            xT_t[:, kt, hf * 512:(hf + 1) * 512],
                            start=(kt == 0), stop=(kt == KT - 1),
                        )
                    nc.vector.tensor_copy(qkT_t[:, nt, hf * 512:(hf + 1) * 512], pm[:])

        def attn_pair(b, qkT_t, v_t, ao_t, p):
            for hf in range(2):
                po = [
                    ps_out.tile([HD + 1, 512], F32, tag="po", name=f"po_{b}_{p}_{hf}_{hs}")
                    for hs in range(2)
                ]
                for kt in range(N // P):
                    sc = ps_sc.tile([P, 2, 512], F32)
                    for hs in range(2):
                        qo = hs * HD
                        nc.tensor.matmul(
                            sc[:, hs, :],
                            qkT_t[qo:qo + HD, KT + p, kt * P:(kt + 1) * P],
                            qkT_t[qo:qo + HD, p, hf * 512:(hf + 1) * 512],
                            start=True, stop=True,
                        )
                    ex = exp_p.tile([P, 2, 512], BF16)
                    nc.scalar.activation(ex[:], sc[:], EXP, scale=SCALE)
                    for hs in range(2):
                        nc.tensor.matmul(
                            po[hs][:],
                            v_t[:, kt, 2 * p + hs, :],
                            ex[:, hs, :],
                            start=(kt == 0), stop=(kt == N // P - 1),
                        )
                for hs in range(2):
                    u_t = smallp.tile([HD, 512], F32, tag="u")
                    nc.vector.tensor_copy(u_t[:], po[hs][0:HD, :])
                    sums_t = smallp.tile([1, 512], F32, tag="sums")
                    nc.vector.tensor_copy(sums_t[:], po[hs][HD:HD + 1, :])
                    rbc = smallp.tile([HD, 512], F32, tag="rbc")
                    nc.gpsimd.partition_broadcast(rbc[:], sums_t[:])
                    rec = smallp.tile([HD, 512], F32, tag="rec")
                    nc.vector.reciprocal_approx_fast(rec[:], rbc[:])
                    nc.vector.tensor_tensor(
                        ao_t[hs * HD:(hs + 1) * HD, p, hf * 512:(hf + 1) * 512],
                        u_t[:], rec[:], mybir.AluOpType.mult,
                    )

        def proj_phase(b, ao_t):
            rows0 = b * N
            for rt in range(N // P):
                y_t = yp.tile([P, D], F32)
                for j in range(2):
                    pm = ps_mm.tile([P, 512], F32)
                    for kt in range(KT):
                        nc.tensor.matmul(
                            pm[:, :384],
                            ao_t[:, kt, rt * P:(rt + 1) * P],
                            wproj_sb[:, kt, j * 384:(j + 1) * 384],
                            start=(kt == 0), stop=(kt == KT - 1),
                        )
                    nc.vector.tensor_add(
                        y_t[:, j * 384:(j + 1) * 384], pm[:, :384],
                        bias_sb[:, j * 384:(j + 1) * 384],
                    )
                nc.sync.dma_start(out[rows0 + rt * P:rows0 + (rt + 1) * P, :], y_t[:])

        NP = H // 2
        # cross-batch software pipeline: b1's v/qk issue during b0's last pairs
        v0 = v_phase(0)
        qkT0 = qkp.tile([P, 2 * KT, N], BF16, tag="qkT", name="qkT_0")
        ao0 = aop.tile([P, KT, N], BF16, tag="ao", name="ao_0")
        qk_pair(0, qkT0, 0)
        qkT1 = qkp.tile([P, 2 * KT, N], BF16, tag="qkT", name="qkT_1")
        ao1 = aop.tile([P, KT, N], BF16, tag="ao", name="ao_1")
        v1 = None
        for p in range(NP):
            attn_pair(0, qkT0, v0, ao0, p)
            if p < NP - 1:
                qk_pair(0, qkT0, p + 1)
            else:
                v1 = v_phase(1)
                qk_pair(1, qkT1, 0)
        proj_phase(0, ao0)
        for p in range(NP):
            attn_pair(1, qkT1, v1, ao1, p)
            if p < NP - 1:
                qk_pair(1, qkT1, p + 1)
        proj_phase(1, ao1)

    nc.compile()
    return nc


_NC_CACHE = None


def _get_nc():
    global _NC_CACHE
    if _NC_CACHE is None:
        _NC_CACHE = build_kernel()
    return _NC_CACHE


def make_in_maps(x, W_qkv, W_proj, b_proj):
    x = np.asarray(x, np.float32)
    wq = np.asarray(W_qkv, np.float32).astype(ml_dtypes.bfloat16)
    wp = np.asarray(W_proj, np.float32).astype(ml_dtypes.bfloat16)
    bias = np.ascontiguousarray(
        np.broadcast_to(np.asarray(b_proj, np.float32), (P, D))
    )
    in_maps = []
    for c in range(NCORES):
        xc = x[BPC * c:BPC * (c + 1)].reshape(ROWS, D).T
        in_maps.append({
            "xT": np.ascontiguousarray(xc).astype(ml_dtypes.bfloat16),
            "wqkv": wq, "wproj": wp, "bias": bias,
        })
    return in_maps


def run(x, W_qkv, W_proj, b_proj, trace=False):
    nc = _get_nc()
    in_maps = make_in_maps(x, W_qkv, W_proj, b_proj)
    res = run_bass_kernel_spmd(nc, in_maps, core_ids=list(range(NCORES)), trace=trace)
    y = np.concatenate(
        [res.results[c]["out"].reshape(BPC, N, D) for c in range(NCORES)], axis=0
    )
    return y.astype(np.float32), res


def kernel(x, W_qkv, W_proj, b_proj):
    y, _ = run(x, W_qkv, W_proj, b_proj, trace=False)
    return y
